# revision 1
# baseline (speedup 1.0000x reference)
"""Trainium2 Bass kernel for a 2-layer GAT (DGL-style) on a random graph.

Strategy (8 NeuronCores, SPMD):
  - dst-node sharding: 392 blocks of 128 dst nodes, LPT-balanced 49 blocks/core.
  - Phase 1 (per core): feat1|er1 = x @ [W1|Vr1] for its own nodes (PE, fp32).
  - AllGather bf16 feat1 table -> every core.
  - Layer-1 edge phase per dst block: dma_gather of feat1[src] rows (bf16,
    512B rows, int16 idx => lo/hi table split at 32768), el1 on-device via
    DVE mult+group-reduce, er1 via indicator-transpose matmul, softmax weights
    w=exp(leakyrelu(el+er)) without max-subtraction (values are small), then
    one accumulating PE matmul per 128-edge chunk: psum += Ind^T @ [w*feat | w].
    Normalize by the summed-w column, bias+relu -> h. Transpose h on PE and
    project: feat2|el2|er2 = h @ [W2|vl2|vr2].
  - AllGather bf16 feat2 table; layer-2 edge phase analogous (1 head, C=40),
    with w folded into the indicator and a ones-column for the denominator.
  - Host assembles per-core [6272,40] outputs via the node permutation.

All graph-structure data (indices, indicator layouts, chunk counts) is
precomputed on the host at kernel() time and baked into inputs / the program.
"""

import sys
sys.path.insert(0, '/opt/trn_rl_repo')

import numpy as np
import ml_dtypes

N_NODES = 50000
N_EDGES = 800000
F_IN = 256
H1, HD = 8, 32
C2 = 40
NEG_SLOPE = 0.2
NCORES = 8
P = 128
BLOCKS_PER_CORE = 49
NODES_PER_CORE = BLOCKS_PER_CORE * P      # 6272
NPAD = NCORES * NODES_PER_CORE            # 50176
NBLOCKS = NPAD // P                       # 392
SPLIT = 32768                             # int16 index limit for dma_gather


def _prep_graph(src, dst):
    """Block assignment, node permutation, per-position chunk layout."""
    src = src.astype(np.int64)
    dst = dst.astype(np.int64)
    blk_of_edge = dst // P
    blk_counts = np.bincount(blk_of_edge, minlength=NBLOCKS)

    # LPT: assign blocks to cores, 49 each, balancing edge totals
    order = np.argsort(-blk_counts)
    core_of_blk = np.zeros(NBLOCKS, np.int64)
    loads = np.zeros(NCORES, np.int64)
    fills = np.zeros(NCORES, np.int64)
    for b in order:
        cands = np.where(fills < BLOCKS_PER_CORE)[0]
        c = cands[np.argmin(loads[cands])]
        core_of_blk[b] = c
        loads[c] += blk_counts[b]
        fills[c] += 1

    # per-core position: sort own blocks by count desc
    pos_of_blk = np.zeros(NBLOCKS, np.int64)
    blocks_at = np.zeros((NCORES, BLOCKS_PER_CORE), np.int64)
    for c in range(NCORES):
        mine = np.where(core_of_blk == c)[0]
        mine = mine[np.argsort(-blk_counts[mine])]
        blocks_at[c] = mine
        pos_of_blk[mine] = np.arange(BLOCKS_PER_CORE)

    # node -> table position
    node_ids = np.arange(NPAD)
    nb = node_ids // P
    pos_of_node = core_of_blk[nb] * NODES_PER_CORE + pos_of_blk[nb] * P + (node_ids % P)

    # group edges by block
    e_order = np.argsort(blk_of_edge, kind='stable')
    e_sorted = e_order
    blk_starts = np.zeros(NBLOCKS + 1, np.int64)
    np.cumsum(blk_counts, out=blk_starts[1:])

    srcpos = pos_of_node[src]
    dstloc = (dst % P).astype(np.int64)

    # lo/hi chunk counts per (core, position)
    n_lo = np.zeros((NCORES, BLOCKS_PER_CORE), np.int64)
    n_hi = np.zeros((NCORES, BLOCKS_PER_CORE), np.int64)
    blk_lo_edges = {}
    blk_hi_edges = {}
    for b in range(NBLOCKS):
        es = e_sorted[blk_starts[b]:blk_starts[b + 1]]
        lo = es[srcpos[es] < SPLIT]
        hi = es[srcpos[es] >= SPLIT]
        blk_lo_edges[b] = lo
        blk_hi_edges[b] = hi
        c, i = core_of_blk[b], pos_of_blk[b]
        n_lo[c, i] = -(-len(lo) // P) if len(lo) else 0
        n_hi[c, i] = -(-len(hi) // P) if len(hi) else 0
    n_lo_max = n_lo.max(axis=0)   # per position
    n_hi_max = n_hi.max(axis=0)
    nb_tot = n_lo_max + n_hi_max
    return dict(core_of_blk=core_of_blk, pos_of_blk=pos_of_blk, blocks_at=blocks_at,
                pos_of_node=pos_of_node, blk_lo=blk_lo_edges, blk_hi=blk_hi_edges,
                srcpos=srcpos, dstloc=dstloc, n_lo_max=n_lo_max, n_hi_max=n_hi_max,
                nb_tot=nb_tot)


def _pack_idx16(vals):
    """dma_gather index layout: [128, n/16], wrapped by 16, replicated x8."""
    v = np.asarray(vals, np.uint16).reshape(-1, 16).T      # [16, n/16]
    return np.tile(v, (8, 1)).view(np.int16)               # [128, n/16]


def _build_core_inputs(g, features, W1, attn_l1, attn_r1, W2, attn_l2, attn_r2, b1, b2):
    bf = ml_dtypes.bfloat16
    NBMAX = int(g['nb_tot'].max())
    CTOT = int(g['nb_tot'].sum())

    Vr1 = np.einsum('kf,f->k...', np.zeros(0), np.zeros(0)) if False else None
    # Wcat1 = [W1 | Vl1 | Vr1]
    Vl1 = np.einsum('khd,hd->kh', W1.reshape(F_IN, H1, HD), attn_l1)
    Vr1 = np.einsum('khd,hd->kh', W1.reshape(F_IN, H1, HD), attn_r1)
    Wcat1 = np.concatenate([W1, Vl1, Vr1], axis=1).astype(np.float32)  # [256, 272]
    vl2 = W2 @ attn_l2[0]
    vr2 = W2 @ attn_r2[0]
    Wcat2 = np.concatenate([W2, vl2[:, None], vr2[:, None]], axis=1).astype(np.float32)  # [256,42]

    alb = np.zeros((P, H1 * HD), np.float32)
    # attn_l1 broadcast tile is built per-problem by caller; placeholder filled there
    iotaR = np.tile(np.arange(P, dtype=np.float32)[None, :], (P, 1))
    iotaR_big = np.tile(iotaR, (1, NBMAX)).astype(ml_dtypes.bfloat16)  # [128, NBMAX*128]
    iotaC_big = np.tile(np.arange(P, dtype=np.int8)[:, None], (1, NBMAX * P))
    b1B = np.tile(b1.astype(np.float32)[None, :], (P, 1))
    b2B = np.tile(b2.astype(np.float32)[None, :], (P, 1))

    feats_pad = np.zeros((NPAD, F_IN), np.float32)
    feats_pad[:N_NODES] = features

    per_core = []
    for c in range(NCORES):
        # xT columns in local node order
        my_nodes = (g['blocks_at'][c][:, None] * P + np.arange(P)[None, :]).reshape(-1)
        xT = feats_pad[my_nodes].T.astype(np.float32).copy()          # [256, 6272]

        idx_cols = []
        dstloc_cols = np.full((P, CTOT), -1.0, ml_dtypes.bfloat16)
        dstlocT = np.full((P, CTOT * P), -1, np.int8)
        ccol = 0
        for i in range(BLOCKS_PER_CORE):
            b = g['blocks_at'][c][i]
            for kind in ('lo', 'hi'):
                nch = int((g['n_lo_max'] if kind == 'lo' else g['n_hi_max'])[i])
                if nch == 0:
                    continue
                es = g['blk_lo' if kind == 'lo' else 'blk_hi'][b]
                sp = g['srcpos'][es] - (0 if kind == 'lo' else SPLIT)
                dl = g['dstloc'][es]
                npad_e = nch * P - len(es)
                sp = np.concatenate([sp, np.zeros(npad_e, np.int64)])
                dl_f = np.concatenate([dl.astype(np.float32),
                                       np.full(npad_e, -1.0, np.float32)])  # exact in bf16
                idx_cols.append(_pack_idx16(sp))
                for ch in range(nch):
                    col = ccol + ch
                    dstloc_cols[:, col] = dl_f[ch * P:(ch + 1) * P]
                    dstlocT[:, col * P:(col + 1) * P] = \
                        dl_f[ch * P:(ch + 1) * P].astype(np.int8)[None, :]
                ccol += nch
        assert ccol == CTOT
        idx_all = np.concatenate(idx_cols, axis=1)                    # [128, CTOT*8]

        per_core.append(dict(xT=xT, idx=idx_all, dstloc=dstloc_cols, dstlocT=dstlocT,
                             Wcat1=Wcat1, Wcat2=Wcat2, iotaR=iotaR_big,
                             iotaC=iotaC_big, b1B=b1B, b2B=b2B))
    return per_core, NBMAX, CTOT


def _build_program(g, NBMAX, CTOT, IDXCOLS):
    import os
    DBG_NOHI = os.environ.get('DBG_NOHI', '0') == '1'     # hi gathers read lo table
    DBG_NOGATH = os.environ.get('DBG_NOGATH', '0') == '1' # skip gathers entirely
    DBG_L1ONLY = os.environ.get('DBG_L1ONLY', '0') == '1' # skip L2 edge phase
    DBG_NQ1 = os.environ.get('DBG_NQ1', '0') == '1'       # single queue
    import concourse.bass as bass
    import concourse.bacc as bacc
    import concourse.mybir as mybir
    import concourse.tile as tile
    from concourse.masks import make_identity

    f32, bf16 = mybir.dt.float32, mybir.dt.bfloat16
    i8, i16 = mybir.dt.int8, mybir.dt.int16
    Alu, Act = mybir.AluOpType, mybir.ActivationFunctionType
    F1 = F_IN                     # 256 features
    F1R = 384                     # L1 table row: feat(256)|el(8)|er(8)|pad (768B)
    F2 = 128                      # L2 table row (feat2|el2|er2|pad)
    n_lo, n_hi = g['n_lo_max'], g['n_hi_max']
    nb_tot = g['nb_tot']

    nc = bacc.Bacc(None, target_bir_lowering=False, debug=False, num_swdge_queues=1 if DBG_NQ1 else 4)

    t_xT = nc.dram_tensor("xT", [F_IN, NODES_PER_CORE], f32, kind="ExternalInput")
    t_idx = nc.dram_tensor("idx", [P, IDXCOLS], i16, kind="ExternalInput")
    t_dstloc = nc.dram_tensor("dstloc", [P, CTOT], bf16, kind="ExternalInput")
    t_dstlocT = nc.dram_tensor("dstlocT", [P, CTOT * P], i8, kind="ExternalInput")
    t_W1 = nc.dram_tensor("Wcat1", [F_IN, 272], f32, kind="ExternalInput")
    t_W2 = nc.dram_tensor("Wcat2", [F_IN, 42], f32, kind="ExternalInput")
    t_alb = nc.dram_tensor("AlB", [P, NBMAX * F1], bf16, kind="ExternalInput")
    t_iotaR = nc.dram_tensor("iotaR", [P, NBMAX * P], bf16, kind="ExternalInput")
    t_iotaC = nc.dram_tensor("iotaC", [P, NBMAX * P], i8, kind="ExternalInput")
    t_b1 = nc.dram_tensor("b1B", [P, F1], f32, kind="ExternalInput")
    t_b2 = nc.dram_tensor("b2B", [P, C2], f32, kind="ExternalInput")
    t_out = nc.dram_tensor("out2", [NODES_PER_CORE, C2], f32, kind="ExternalOutput")

    with tile.TileContext(nc) as tc:
        with tc.tile_pool(name="dram", bufs=1, space="DRAM") as dram, \
             tc.tile_pool(name="const", bufs=1) as cst, \
             tc.tile_pool(name="resid", bufs=1) as res, \
             tc.tile_pool(name="work", bufs=3) as wk, \
             tc.tile_pool(name="gath", bufs=4) as gp, \
             tc.tile_pool(name="indp", bufs=3) as indp, \
             tc.tile_pool(name="rhsp", bufs=4) as rhsp, \
             tc.tile_pool(name="ps_agg", bufs=2, space="PSUM") as ps_agg, \
             tc.tile_pool(name="ps_er", bufs=2, space="PSUM") as ps_er, \
             tc.tile_pool(name="ps_t", bufs=2, space="PSUM") as ps_t, \
             tc.tile_pool(name="ps_f2", bufs=2, space="PSUM") as ps_f2:
            NBH = NBMAX * H1

            T1_local = dram.tile([NODES_PER_CORE, F1R], bf16)
            T1_full = dram.tile([NPAD, F1R], bf16, addr_space="Shared")
            T2_local = dram.tile([NODES_PER_CORE, F2], bf16)
            T2_full = dram.tile([NPAD, F2], bf16, addr_space="Shared")
            T1_hi = dram.tile([NPAD - SPLIT, F1R], bf16)
            T2_hi = dram.tile([NPAD - SPLIT, F2], bf16)

            # ---- constants ----
            iotaR = cst.tile([P, NBMAX * P], bf16)
            nc.sync.dma_start(iotaR[:], t_iotaR[:])
            iotaC = cst.tile([P, NBMAX * P], i8)
            nc.sync.dma_start(iotaC[:], t_iotaC[:])
            b1B = cst.tile([P, F1], f32)
            nc.sync.dma_start(b1B[:], t_b1[:])
            b2B = cst.tile([P, C2], f32)
            nc.sync.dma_start(b2B[:], t_b2[:])
            Wc2 = cst.tile([P, 2, 42], f32)
            nc.sync.dma_start(Wc2[:, 0, :], t_W2[0:128, :])
            nc.sync.dma_start(Wc2[:, 1, :], t_W2[128:256, :])
            ident = cst.tile([P, P], f32)
            make_identity(nc, ident[:])
            alpha = cst.tile([P, 1], f32)
            nc.vector.memset(alpha[:], NEG_SLOPE)
            er1_sb = res.tile([P, BLOCKS_PER_CORE * H1], bf16)
            er2_sb = res.tile([P, BLOCKS_PER_CORE], bf16)
            idx_sb = res.tile([P, IDXCOLS], i16)
            nc.sync.dma_start(idx_sb[:], t_idx[:])
            dstloc_sb = res.tile([P, CTOT], bf16)
            nc.sync.dma_start(dstloc_sb[:], t_dstloc[:])


            qctr = [0]
            def emit_gathers(Gt, table_lo, table_hi, nlo, nhi, nbi, icol, elem):
                # returns new icol; splits lo/hi into 2 sub-gathers each for queue overlap
                def emit(dst_c0, dst_c1, table, ic):
                    n = dst_c1 - dst_c0
                    if n <= 0:
                        return ic
                    half = (n + 1) // 2
                    for (a, b) in (((dst_c0, dst_c0 + half)), ((dst_c0 + half, dst_c1))):
                        m = b - a
                        if m <= 0:
                            continue
                        q = qctr[0] % 4
                        qctr[0] += 1
                        nc.gpsimd.dma_gather(
                            Gt[:, a:b, :], table,
                            idx_sb[:, ic:ic + m * 8], m * P, m * P, elem,
                            single_packet=False, queue_num=q)
                        ic += m * 8
                    return ic
                icol = emit(0, nlo, table_lo, icol)
                icol = emit(nlo, nbi, table_hi, icol)
                return icol
            # ---- phase 1: feat1|er1 = x @ [W1|Vr1] ----
            with tc.tile_pool(name="p1", bufs=3) as p1, \
                 tc.tile_pool(name="p1w", bufs=1) as p1w:
                w1a = p1w.tile([P, 272], f32)
                nc.sync.dma_start(w1a[:], t_W1[0:128, :])
                w1b = p1w.tile([P, 272], f32)
                nc.sync.dma_start(w1b[:], t_W1[128:256, :])
                for b in range(BLOCKS_PER_CORE):
                    sl = slice(b * P, (b + 1) * P)
                    xt0 = p1.tile([P, P], f32, tag="xt0")
                    nc.sync.dma_start(xt0[:], t_xT[0:128, sl])
                    xt1 = p1.tile([P, P], f32, tag="xt1")
                    nc.sync.dma_start(xt1[:], t_xT[128:256, sl])
                    acc = ps_t.tile([P, 272], f32, space="PSUM", tag="htp")
                    nc.tensor.matmul(acc[:], lhsT=xt0[:], rhs=w1a[:], start=True, stop=False)
                    nc.tensor.matmul(acc[:], lhsT=xt1[:], rhs=w1b[:], start=False, stop=True)
                    fb = wk.tile([P, F1R], bf16, tag="p1out")
                    nc.vector.tensor_copy(out=fb[:, 0:272], in_=acc[:])
                    nc.sync.dma_start(T1_local[sl, :], fb[:])
                    nc.vector.tensor_copy(out=er1_sb[:, b * H1:(b + 1) * H1],
                                          in_=acc[:, 264:272])

            nc.gpsimd.collective_compute(
                "AllGather", mybir.AluOpType.bypass,
                replica_groups=[list(range(NCORES))],
                ins=[T1_local[:]], outs=[T1_full[:]])
            nc.sync.dma_start(T1_hi[:], T1_full[SPLIT:NPAD, :])

            # ---- layer 1 edge phase ----
            ccol = 0
            icol = 0
            for b in range(BLOCKS_PER_CORE):
                nbi = int(nb_tot[b])
                nlo, nhi = int(n_lo[b]), int(n_hi[b])
                G = gp.tile([P, NBMAX, F1R], bf16, tag="g1")
                icol = emit_gathers(G, T1_full[0:SPLIT, :], T1_hi[:], nlo, nhi, nbi, icol, F1R)

                dT = indp.tile([P, NBMAX * P], i8, tag="dT")
                nc.sync.dma_start(dT[:, 0:nbi * P], t_dstlocT[:, ccol * P:(ccol + nbi) * P])
                ind = indp.tile([P, NBMAX * P], bf16, tag="ind")
                nc.vector.tensor_tensor(
                    out=ind[:, 0:nbi * P],
                    in0=dstloc_sb[:, ccol:ccol + nbi, None].to_broadcast([P, nbi, P]),
                    in1=iotaR[:, 0:nbi * P], op=Alu.is_equal)
                indT = indp.tile([P, NBMAX * P], bf16, tag="indT")
                nc.vector.tensor_tensor(
                    out=indT[:, 0:nbi * P], in0=iotaC[:, 0:nbi * P],
                    in1=dT[:, 0:nbi * P], op=Alu.is_equal)

                # er per chunk via IndT matmul
                ers = ps_er.tile([P, NBH], f32, space="PSUM", tag="ers")
                for c in range(nbi):
                    nc.tensor.matmul(ers[:, c * H1:(c + 1) * H1],
                                     lhsT=indT[:, c * P:(c + 1) * P],
                                     rhs=er1_sb[:, b * H1:(b + 1) * H1],
                                     start=True, stop=True)
                ee = wk.tile([P, NBMAX * H1], f32, tag="ee")
                nc.vector.tensor_tensor(
                    out=ee[:, 0:nbi * H1].rearrange("p (a b) -> p a b", b=H1),
                    in0=G[:, 0:nbi, F1:F1 + H1],
                    in1=ers[:, 0:nbi * H1].rearrange("p (a b) -> p a b", b=H1),
                    op=Alu.add)
                nc.scalar.activation(ee[:, 0:nbi * H1], ee[:, 0:nbi * H1],
                                     Act.Prelu, alpha=alpha[:, :1])
                # rhs_all[:, c, 0:256] = w_c (bcast 32) * feat_c ; [:, c, 256:264] = w_c
                w = wk.tile([P, NBMAX * H1], bf16, tag="w")
                nc.scalar.activation(w[:, 0:nbi * H1], ee[:, 0:nbi * H1], Act.Exp)
                rhs_all = wk.tile([P, NBMAX, 264], bf16, tag="rhsall")
                nc.scalar.copy(
                    out=rhs_all[:, 0:nbi, F1:264],
                    in_=w[:, 0:nbi * H1].rearrange("p (a b) -> p a b", b=H1))
                nc.vector.tensor_tensor(
                    out=rhs_all[:, 0:nbi, 0:F1].rearrange("p a (h d) -> p a h d", d=HD),
                    in0=G[:, 0:nbi, 0:F1].rearrange("p a (h d) -> p a h d", d=HD),
                    in1=w[:, 0:nbi * H1].rearrange("p (a b) -> p a b", b=H1)[:, :, :, None]
                        .to_broadcast([P, nbi, H1, HD]),
                    op=Alu.mult)
                acc = ps_agg.tile([P, 264], f32, space="PSUM", tag="agg")
                for c in range(nbi):
                    nc.tensor.matmul(acc[:], lhsT=ind[:, c * P:(c + 1) * P],
                                     rhs=rhs_all[:, c, :],
                                     start=(c == 0), stop=(c == nbi - 1))

                # normalize + bias + relu -> h
                den = wk.tile([P, H1], f32, tag="den")
                nc.vector.tensor_scalar_max(den[:], acc[:, F1:264], 1e-30)
                rec = wk.tile([P, H1], f32, tag="rec")
                nc.vector.reciprocal(rec[:], den[:])
                h = wk.tile([P, F1], f32, tag="h")
                nc.vector.tensor_tensor(out=h[:], in0=acc[:, 0:F1],
                                        in1=rec[:, :, None].to_broadcast([P, H1, HD]),
                                        op=Alu.mult)
                nc.vector.tensor_tensor(out=h[:], in0=h[:], in1=b1B[:], op=Alu.add)
                nc.vector.tensor_scalar_max(h[:], h[:], 0.0)

                # feat2|el2|er2 = h @ Wcat2 (transpose h on PE first)
                f2 = ps_f2.tile([P, 42], f32, space="PSUM", tag="f2")
                for j in range(2):
                    ht_ps = ps_t.tile([P, 272], f32, space="PSUM", tag="htp")
                    nc.tensor.transpose(ht_ps[:, 0:P], h[:, j * P:(j + 1) * P], ident[:])
                    ht = wk.tile([P, P], f32, tag="ht")
                    if j == 0:
                        nc.scalar.copy(out=ht[:], in_=ht_ps[:, 0:P])
                    else:
                        nc.vector.tensor_copy(out=ht[:], in_=ht_ps[:, 0:P])
                    nc.tensor.matmul(f2[:], lhsT=ht[:], rhs=Wc2[:, j, :],
                                     start=(j == 0), stop=(j == 1))
                t2r = wk.tile([P, F2], bf16, tag="t2r")
                nc.vector.tensor_copy(out=t2r[:, 0:42], in_=f2[:])
                nc.vector.memset(t2r[:, 42:F2], 0)
                nc.sync.dma_start(T2_local[b * P:(b + 1) * P, :], t2r[:])
                nc.vector.tensor_copy(out=er2_sb[:, b:b + 1], in_=f2[:, 41:42])
                ccol += nbi

            nc.gpsimd.collective_compute(
                "AllGather", mybir.AluOpType.bypass,
                replica_groups=[list(range(NCORES))],
                ins=[T2_local[:]], outs=[T2_full[:]])
            nc.sync.dma_start(T2_hi[:], T2_full[SPLIT:NPAD, :])

            # ---- layer 2 edge phase ----
            ccol = 0
            icol = 0
            for b in range(0 if DBG_L1ONLY else BLOCKS_PER_CORE):
                nbi = int(nb_tot[b])
                nlo, nhi = int(n_lo[b]), int(n_hi[b])
                G2 = gp.tile([P, NBMAX, F2], bf16, tag="g2")
                icol = emit_gathers(G2, T2_full[0:SPLIT, :], T2_hi[:], nlo, nhi, nbi, icol, F2)

                dT = indp.tile([P, NBMAX * P], i8, tag="dT")
                nc.sync.dma_start(dT[:, 0:nbi * P], t_dstlocT[:, ccol * P:(ccol + nbi) * P])
                ind = indp.tile([P, NBMAX * P], bf16, tag="ind")
                nc.vector.tensor_tensor(
                    out=ind[:, 0:nbi * P],
                    in0=dstloc_sb[:, ccol:ccol + nbi, None].to_broadcast([P, nbi, P]),
                    in1=iotaR[:, 0:nbi * P], op=Alu.is_equal)
                indT = indp.tile([P, NBMAX * P], bf16, tag="indT")
                nc.vector.tensor_tensor(
                    out=indT[:, 0:nbi * P], in0=iotaC[:, 0:nbi * P],
                    in1=dT[:, 0:nbi * P], op=Alu.is_equal)

                ers = ps_er.tile([P, NBH], f32, space="PSUM", tag="ers")
                for c in range(nbi):
                    nc.tensor.matmul(ers[:, c:c + 1],
                                     lhsT=indT[:, c * P:(c + 1) * P],
                                     rhs=er2_sb[:, b:b + 1], start=True, stop=True)
                ee = wk.tile([P, NBMAX], f32, tag="ee2")
                nc.vector.tensor_tensor(
                    out=ee[:, 0:nbi],
                    in0=G2[:, 0:nbi, 40:41].rearrange("p a b -> p (a b)"),
                    in1=ers[:, 0:nbi], op=Alu.add)
                nc.scalar.activation(ee[:, 0:nbi], ee[:, 0:nbi], Act.Prelu,
                                     alpha=alpha[:, :1])
                w2 = wk.tile([P, NBMAX], bf16, tag="w2")
                nc.scalar.activation(w2[:, 0:nbi], ee[:, 0:nbi], Act.Exp)

                iw = wk.tile([P, NBMAX * P], bf16, tag="iw")
                nc.vector.tensor_tensor(
                    out=iw[:, 0:nbi * P].rearrange("p (a b) -> p a b", b=P),
                    in0=ind[:, 0:nbi * P].rearrange("p (a b) -> p a b", b=P),
                    in1=w2[:, 0:nbi, None].to_broadcast([P, nbi, P]), op=Alu.mult)
                rhs2 = wk.tile([P, NBMAX, 41], bf16, tag="rhs2a")
                nc.vector.memset(rhs2[:, 0:nbi, :], 1.0)
                nc.vector.tensor_copy(out=rhs2[:, 0:nbi, 0:C2], in_=G2[:, 0:nbi, 0:C2])
                acc = ps_agg.tile([P, 264], f32, space="PSUM", tag="agg")
                for c in range(nbi):
                    nc.tensor.matmul(acc[:, 0:41], lhsT=iw[:, c * P:(c + 1) * P],
                                     rhs=rhs2[:, c, :],
                                     start=(c == 0), stop=(c == nbi - 1))

                den = wk.tile([P, 1], f32, tag="den2")
                nc.vector.tensor_scalar_max(den[:], acc[:, C2:41], 1e-30)
                rec = wk.tile([P, 1], f32, tag="rec2")
                nc.vector.reciprocal(rec[:], den[:])
                o = wk.tile([P, C2], f32, tag="o")
                nc.vector.tensor_tensor(out=o[:], in0=acc[:, 0:C2],
                                        in1=rec[:, :1].to_broadcast([P, C2]), op=Alu.mult)
                nc.vector.tensor_tensor(out=o[:], in0=o[:], in1=b2B[:], op=Alu.add)
                nc.sync.dma_start(t_out[b * P:(b + 1) * P, :], o[:])
                ccol += nbi
            if DBG_L1ONLY:
                z = wk.tile([P, C2], f32, tag="o")
                nc.vector.memset(z[:], 0)
                for b in range(BLOCKS_PER_CORE):
                    nc.sync.dma_start(t_out[b * P:(b + 1) * P, :], z[:])

    nc.compile()
    return nc


def kernel(features, src, dst, W1, attn_l1, attn_r1, b1, W2, attn_l2, attn_r2, b2):
    from concourse import bass_utils

    features = np.asarray(features, np.float32)
    src = np.asarray(src)
    dst = np.asarray(dst)
    W1 = np.asarray(W1, np.float32)
    attn_l1 = np.asarray(attn_l1, np.float32)
    attn_r1 = np.asarray(attn_r1, np.float32)
    b1 = np.asarray(b1, np.float32)
    W2 = np.asarray(W2, np.float32)
    attn_l2 = np.asarray(attn_l2, np.float32)
    attn_r2 = np.asarray(attn_r2, np.float32)
    b2 = np.asarray(b2, np.float32)

    g = _prep_graph(src, dst)
    per_core, NBMAX, CTOT = _build_core_inputs(
        g, features, W1, attn_l1, attn_r1, W2, attn_l2, attn_r2, b1, b2)

    # attn_l1 broadcast tile (bf16), tiled NBMAX times
    alb_one = np.tile(attn_l1.reshape(1, H1 * HD), (P, 1)).astype(np.float32)
    alb_big = np.tile(alb_one, (1, NBMAX)).astype(ml_dtypes.bfloat16)
    for pc in per_core:
        pc['AlB'] = alb_big

    IDXCOLS = per_core[0]['idx'].shape[1]
    nc = _build_program(g, NBMAX, CTOT, IDXCOLS)

    in_maps = []
    for pc in per_core:
        in_maps.append({
            "xT": pc['xT'], "idx": pc['idx'], "dstloc": pc['dstloc'],
            "dstlocT": pc['dstlocT'], "Wcat1": pc['Wcat1'], "Wcat2": pc['Wcat2'],
            "AlB": np.asarray(pc['AlB']), "iotaR": pc['iotaR'], "iotaC": pc['iotaC'],
            "b1B": pc['b1B'], "b2B": pc['b2B'],
        })

    res = bass_utils.run_bass_kernel_spmd(
        nc, in_maps, core_ids=list(range(NCORES)),
        trace=bool(int(__import__('os').environ.get('KTRACE', '0'))))
    kernel.last_result = res

    out = np.zeros((N_NODES, C2), np.float32)
    for c in range(NCORES):
        oc = res.results[c]["out2"]
        for i in range(BLOCKS_PER_CORE):
            b = g['blocks_at'][c][i]
            lo = b * P
            hi = min(lo + P, N_NODES)
            if hi > lo:
                out[lo:hi] = oc[i * P: i * P + (hi - lo)]
    return out



# revision 3
# speedup vs baseline: 1.1540x; 1.1540x over previous
"""Trainium2 Bass kernel for a 2-layer GAT (DGL-style) on a random graph.

v2 design (8 NeuronCores, SPMD, dst-node sharding):
  - 392 blocks of 128 dst nodes, LPT-balanced 49 blocks/core.
  - Node table positions interleave by local half: per core, blocks 0..24
    feed half-table A (row = c*3200 + i), blocks 25..48 feed half-table B
    (row = c*3072 + (i-3200)).  Both tables < 32768 rows -> int16 gather
    indices everywhere, no hi-table copies.
  - Phase 1 (bf16): feat1|el1|er1 = x @ [W1|Vl1|Vr1]; feat1 columns stored
    in (d,h) transposed order so the edge-phase w*feat multiply hits DVE 2x
    mode.  Local rows DMA'd to T1_localA/B; two chunked AllGathers (A after
    block 24 overlaps the rest of phase 1, B at the end).
  - L1 edge phase per dst block: 2 dma_gathers (tables A/B, 768B rows:
    256 feat (d,h) + 8 el + pad).  Indicator matrices ind/indT are
    PREcomputed on the host as fp8 0/1 and DMA'd (one interleaved tensor),
    eliminating the DVE is_equal builds.  er per edge via per-chunk
    indT^T @ er1_sb matmuls; w = exp(leakyrelu(el+er)); rhs = [w*feat | w]
    (2x DVE); psum += ind^T @ rhs per chunk; normalize, bias, relu;
    feat2|el2|er2 = h @ Wcat2 via PE transpose; rows to T2_localA/B.
  - Two chunked AllGathers for T2 (A overlaps L1 blocks 25..48).
  - L2 edge phase: same structure, 1 head, C=40, 256B rows, rhs-side
    weighting rhs2 = [w2*feat2 | w2] with plain fp8 ind as lhsT.
  - Host assembles per-core [6272,40] outputs via the block permutation.

All graph-structure data (indices, indicators, chunk counts) is precomputed
on the host at kernel() time and baked into inputs / the program.
"""

import sys
sys.path.insert(0, '/opt/trn_rl_repo')

import os
import numpy as np
import ml_dtypes

N_NODES = 50000
N_EDGES = 800000
F_IN = 256
H1, HD = 8, 32
C2 = 40
NEG_SLOPE = 0.2
NCORES = 8
P = 128
BLOCKS_PER_CORE = 49
NODES_PER_CORE = BLOCKS_PER_CORE * P      # 6272
NPAD = NCORES * NODES_PER_CORE            # 50176
NBLOCKS = NPAD // P                       # 392
ABLOCKS = 25                              # blocks in half A
AROWS = ABLOCKS * P                       # 3200 per core
BROWS = NODES_PER_CORE - AROWS            # 3072 per core
ATOT = NCORES * AROWS                     # 25600  (< 32768)
BTOT = NCORES * BROWS                     # 24576  (< 32768)
F1R = 384                                 # L1 table row cols (768B)
F2R = 128                                 # L2 table row cols (256B)

# (d,h) permutation: new col d*H1+h  <- old col h*HD+d
_PERM_DH = np.arange(F_IN).reshape(H1, HD).T.reshape(-1)   # len 256


def _prep_graph(src, dst):
    """Block assignment, node->table-row map, per-position chunk layout."""
    src = src.astype(np.int64)
    dst = dst.astype(np.int64)
    blk_of_edge = dst // P
    blk_counts = np.bincount(blk_of_edge, minlength=NBLOCKS)

    # LPT: assign blocks to cores, 49 each, balancing edge totals
    order = np.argsort(-blk_counts)
    core_of_blk = np.zeros(NBLOCKS, np.int64)
    loads = np.zeros(NCORES, np.int64)
    fills = np.zeros(NCORES, np.int64)
    for b in order:
        cands = np.where(fills < BLOCKS_PER_CORE)[0]
        c = cands[np.argmin(loads[cands])]
        core_of_blk[b] = c
        loads[c] += blk_counts[b]
        fills[c] += 1

    # per-core position: sort own blocks by count desc
    pos_of_blk = np.zeros(NBLOCKS, np.int64)
    blocks_at = np.zeros((NCORES, BLOCKS_PER_CORE), np.int64)
    for c in range(NCORES):
        mine = np.where(core_of_blk == c)[0]
        mine = mine[np.argsort(-blk_counts[mine])]
        blocks_at[c] = mine
        pos_of_blk[mine] = np.arange(BLOCKS_PER_CORE)

    # node -> (half, table row)
    node_ids = np.arange(NPAD)
    nb = node_ids // P
    local_i = pos_of_blk[nb] * P + (node_ids % P)          # 0..6271
    ncore = core_of_blk[nb]
    in_a = local_i < AROWS
    row = np.where(in_a, ncore * AROWS + local_i,
                   ncore * BROWS + (local_i - AROWS))

    src_in_a = in_a[src]
    src_row = row[src]
    dstloc = (dst % P).astype(np.int64)

    # group edges by block
    e_order = np.argsort(blk_of_edge, kind='stable')
    blk_starts = np.zeros(NBLOCKS + 1, np.int64)
    np.cumsum(blk_counts, out=blk_starts[1:])

    # per block: A-edges then B-edges; chunk counts
    n_a = np.zeros((NCORES, BLOCKS_PER_CORE), np.int64)
    n_b = np.zeros((NCORES, BLOCKS_PER_CORE), np.int64)
    blk_a = {}
    blk_b = {}
    for b in range(NBLOCKS):
        es = e_order[blk_starts[b]:blk_starts[b + 1]]
        a = es[src_in_a[es]]
        bb = es[~src_in_a[es]]
        blk_a[b] = a
        blk_b[b] = bb
        c, i = core_of_blk[b], pos_of_blk[b]
        n_a[c, i] = -(-len(a) // P) if len(a) else 0
        n_b[c, i] = -(-len(bb) // P) if len(bb) else 0
    n_a_max = n_a.max(axis=0)
    n_b_max = n_b.max(axis=0)
    nb_tot = n_a_max + n_b_max
    return dict(core_of_blk=core_of_blk, pos_of_blk=pos_of_blk,
                blocks_at=blocks_at, blk_a=blk_a, blk_b=blk_b,
                src_row=src_row, dstloc=dstloc,
                n_a_max=n_a_max, n_b_max=n_b_max, nb_tot=nb_tot)


def _pack_idx16(vals):
    """dma_gather index layout: [128, n/16], wrapped by 16, replicated x8."""
    v = np.asarray(vals, np.uint16).reshape(-1, 16).T      # [16, n/16]
    return np.tile(v, (8, 1)).view(np.int16)               # [128, n/16]


def _build_core_inputs(g, features, W1, attn_l1, attn_r1, W2, attn_l2,
                       attn_r2, b1, b2):
    fp8 = ml_dtypes.float8_e4m3
    NBMAX = int(g['nb_tot'].max())
    CTOT = int(g['nb_tot'].sum())

    # Wcat1 = [W1(d,h-permuted) | Vl1 | Vr1]  (bf16)
    Vl1 = np.einsum('khd,hd->kh', W1.reshape(F_IN, H1, HD), attn_l1)
    Vr1 = np.einsum('khd,hd->kh', W1.reshape(F_IN, H1, HD), attn_r1)
    Wcat1 = np.concatenate([W1[:, _PERM_DH], Vl1, Vr1], axis=1)
    Wcat1 = Wcat1.astype(ml_dtypes.bfloat16)               # [256, 272]
    # Wcat2 rows permuted to (d,h) order to match h's layout
    vl2 = W2 @ attn_l2[0]
    vr2 = W2 @ attn_r2[0]
    Wcat2 = np.concatenate([W2, vl2[:, None], vr2[:, None]], axis=1)
    Wcat2 = Wcat2[_PERM_DH].astype(ml_dtypes.bfloat16)     # [256, 42]

    b1B = np.tile(b1[_PERM_DH].astype(np.float32)[None, :], (P, 1))
    b2B = np.tile(b2.astype(np.float32)[None, :], (P, 1))

    feats_pad = np.zeros((NPAD, F_IN), np.float32)
    feats_pad[:N_NODES] = features

    per_core = []
    for c in range(NCORES):
        my_nodes = (g['blocks_at'][c][:, None] * P
                    + np.arange(P)[None, :]).reshape(-1)
        xT = feats_pad[my_nodes].T.astype(ml_dtypes.bfloat16).copy()  # [256,6272]

        idx_cols = []
        onehot = np.zeros((CTOT, P, P), np.uint8)   # [chunk, edge, dstloc]
        ccol = 0
        for i in range(BLOCKS_PER_CORE):
            b = g['blocks_at'][c][i]
            for kind in ('a', 'b'):
                nch = int((g['n_a_max'] if kind == 'a' else g['n_b_max'])[i])
                if nch == 0:
                    continue
                es = g['blk_a' if kind == 'a' else 'blk_b'][b]
                rows = g['src_row'][es]
                dl = g['dstloc'][es]
                npad_e = nch * P - len(es)
                rows = np.concatenate([rows, np.zeros(npad_e, np.int64)])
                idx_cols.append(_pack_idx16(rows))
                ne = len(es)
                if ne:
                    ch_idx = ccol + np.arange(ne) // P
                    onehot[ch_idx, np.arange(ne) % P, dl] = 1
                ccol += nch
        assert ccol == CTOT
        idx_all = np.concatenate(idx_cols, axis=1)          # [128, CTOT*8]

        # interleaved [ind | indT] per chunk: [128, CTOT, 256] fp8
        comb = np.zeros((P, CTOT, 2 * P), np.uint8)
        comb[:, :, :P] = onehot.transpose(1, 0, 2)          # ind[e,c,j]
        comb[:, :, P:] = onehot.transpose(2, 0, 1)          # indT[p,c,e]
        comb = comb.astype(fp8)

        per_core.append(dict(xT=xT, idx=idx_all, indc=comb.reshape(P, -1),
                             Wcat1=np.asarray(Wcat1), Wcat2=np.asarray(Wcat2),
                             b1B=b1B, b2B=b2B))
    return per_core, NBMAX, CTOT


def _build_program(g, NBMAX, CTOT, IDXCOLS):
    import concourse.bass as bass
    import concourse.bacc as bacc
    import concourse.mybir as mybir
    import concourse.tile as tile
    from concourse.masks import make_identity

    f32, bf16 = mybir.dt.float32, mybir.dt.bfloat16
    i16, f8 = mybir.dt.int16, mybir.dt.float8e4
    Alu, Act = mybir.AluOpType, mybir.ActivationFunctionType
    n_a, n_b = g['n_a_max'], g['n_b_max']
    nb_tot = g['nb_tot']
    NBH = NBMAX * H1

    nc = bacc.Bacc(None, target_bir_lowering=False, debug=False,
                   num_swdge_queues=4)

    t_xT = nc.dram_tensor("xT", [F_IN, NODES_PER_CORE], bf16,
                          kind="ExternalInput")
    t_idx = nc.dram_tensor("idx", [P, IDXCOLS], i16, kind="ExternalInput")
    t_ind = nc.dram_tensor("indc", [P, CTOT * 2 * P], f8, kind="ExternalInput")
    t_W1 = nc.dram_tensor("Wcat1", [F_IN, 272], bf16, kind="ExternalInput")
    t_W2 = nc.dram_tensor("Wcat2", [F_IN, 42], bf16, kind="ExternalInput")
    t_b1 = nc.dram_tensor("b1B", [P, F_IN], f32, kind="ExternalInput")
    t_b2 = nc.dram_tensor("b2B", [P, C2], f32, kind="ExternalInput")
    t_out = nc.dram_tensor("out2", [NODES_PER_CORE, C2], f32,
                           kind="ExternalOutput")

    with tile.TileContext(nc) as tc:
        with tc.tile_pool(name="dram", bufs=1, space="DRAM") as dram, \
             tc.tile_pool(name="const", bufs=1) as cst, \
             tc.tile_pool(name="resid", bufs=1) as res, \
             tc.tile_pool(name="work", bufs=3) as wk, \
             tc.tile_pool(name="gath", bufs=3) as gp, \
             tc.tile_pool(name="indp", bufs=3) as indp, \
             tc.tile_pool(name="ps_agg", bufs=2, space="PSUM") as ps_agg, \
             tc.tile_pool(name="ps_er", bufs=2, space="PSUM") as ps_er, \
             tc.tile_pool(name="ps_t", bufs=2, space="PSUM") as ps_t, \
             tc.tile_pool(name="ps_f2", bufs=2, space="PSUM") as ps_f2:

            T1_localA = dram.tile([AROWS, F1R], bf16)
            T1_localB = dram.tile([BROWS, F1R], bf16)
            T1A = dram.tile([ATOT, F1R], bf16, addr_space="Shared")
            T1B = dram.tile([BTOT, F1R], bf16, addr_space="Shared")
            T2_localA = dram.tile([AROWS, F2R], bf16)
            T2_localB = dram.tile([BROWS, F2R], bf16)
            T2A = dram.tile([ATOT, F2R], bf16, addr_space="Shared")
            T2B = dram.tile([BTOT, F2R], bf16, addr_space="Shared")

            # ---- constants ----
            b1B = cst.tile([P, F_IN], f32)
            nc.sync.dma_start(b1B[:], t_b1[:])
            b2B = cst.tile([P, C2], f32)
            nc.sync.dma_start(b2B[:], t_b2[:])
            Wc2 = cst.tile([P, 2, 42], bf16)
            nc.sync.dma_start(Wc2[:, 0, :], t_W2[0:128, :])
            nc.sync.dma_start(Wc2[:, 1, :], t_W2[128:256, :])
            ident = cst.tile([P, P], f32)
            make_identity(nc, ident[:])
            alpha = cst.tile([P, 1], f32)
            nc.vector.memset(alpha[:], NEG_SLOPE)
            er1_sb = res.tile([P, BLOCKS_PER_CORE * H1], bf16)
            er2_sb = res.tile([P, BLOCKS_PER_CORE], bf16)
            idx_sb = res.tile([P, IDXCOLS], i16)
            nc.sync.dma_start(idx_sb[:], t_idx[:])

            qctr = [0]

            def emit_gathers(Gt, tblA, tblB, na, nbk, icol, elem):
                for (n0, n1, tbl) in ((0, na, tblA), (na, na + nbk, tblB)):
                    m = n1 - n0
                    if m <= 0:
                        continue
                    q = qctr[0] % 4
                    qctr[0] += 1
                    nc.gpsimd.dma_gather(
                        Gt[:, n0:n1, :], tbl,
                        idx_sb[:, icol:icol + m * 8], m * P, m * P, elem,
                        single_packet=False, queue_num=q)
                    icol += m * 8
                return icol

            def allgather(t_in, t_out):
                nc.gpsimd.collective_compute(
                    "AllGather", mybir.AluOpType.bypass,
                    replica_groups=[list(range(NCORES))],
                    ins=[t_in[:]], outs=[t_out[:]])

            # ---- phase 1: feat1(d,h)|el1|er1 = x @ [W1p|Vl1|Vr1] ----
            with tc.tile_pool(name="p1", bufs=3) as p1, \
                 tc.tile_pool(name="p1w", bufs=1) as p1w:
                w1a = p1w.tile([P, 272], bf16)
                nc.sync.dma_start(w1a[:], t_W1[0:128, :])
                w1b = p1w.tile([P, 272], bf16)
                nc.sync.dma_start(w1b[:], t_W1[128:256, :])
                for b in range(BLOCKS_PER_CORE):
                    sl = slice(b * P, (b + 1) * P)
                    xt0 = p1.tile([P, P], bf16, tag="xt0")
                    nc.sync.dma_start(xt0[:], t_xT[0:128, sl])
                    xt1 = p1.tile([P, P], bf16, tag="xt1")
                    nc.sync.dma_start(xt1[:], t_xT[128:256, sl])
                    acc = ps_t.tile([P, 272], f32, space="PSUM", tag="htp")
                    nc.tensor.matmul(acc[:], lhsT=xt0[:], rhs=w1a[:],
                                     start=True, stop=False)
                    nc.tensor.matmul(acc[:], lhsT=xt1[:], rhs=w1b[:],
                                     start=False, stop=True)
                    fb = wk.tile([P, F1R], bf16, tag="p1out")
                    nc.vector.tensor_copy(out=fb[:, 0:264], in_=acc[:, 0:264])
                    if b < ABLOCKS:
                        nc.sync.dma_start(T1_localA[sl, :], fb[:])
                    else:
                        sl2 = slice((b - ABLOCKS) * P, (b - ABLOCKS + 1) * P)
                        nc.sync.dma_start(T1_localB[sl2, :], fb[:])
                    nc.vector.tensor_copy(out=er1_sb[:, b * H1:(b + 1) * H1],
                                          in_=acc[:, 264:272])
                    if b == ABLOCKS - 1:
                        allgather(T1_localA, T1A)
            allgather(T1_localB, T1B)

            # ---- layer 1 edge phase ----
            ccol = 0
            icol = 0
            for b in range(BLOCKS_PER_CORE):
                nbi = int(nb_tot[b])
                na, nbk = int(n_a[b]), int(n_b[b])
                G = gp.tile([P, NBMAX, F1R], bf16, tag="g1")
                icol = emit_gathers(G, T1A[:], T1B[:], na, nbk, icol, F1R)

                ic = indp.tile([P, NBMAX, 2 * P], f8, tag="ind")
                nc.sync.dma_start(ic[:, 0:nbi, :],
                                  t_ind[:, ccol * 2 * P:(ccol + nbi) * 2 * P])

                ers = ps_er.tile([P, NBH], f32, space="PSUM", tag="ers")
                for c in range(nbi):
                    nc.tensor.matmul(ers[:, c * H1:(c + 1) * H1],
                                     lhsT=ic[:, c, P:2 * P],
                                     rhs=er1_sb[:, b * H1:(b + 1) * H1],
                                     start=True, stop=True)
                ee = wk.tile([P, NBMAX * H1], f32, tag="ee")
                nc.vector.tensor_tensor(
                    out=ee[:, 0:nbi * H1].rearrange("p (a h) -> p a h", h=H1),
                    in0=G[:, 0:nbi, F_IN:F_IN + H1],
                    in1=ers[:, 0:nbi * H1].rearrange("p (a h) -> p a h", h=H1),
                    op=Alu.add)
                nc.scalar.activation(ee[:, 0:nbi * H1], ee[:, 0:nbi * H1],
                                     Act.Prelu, alpha=alpha[:, :1])
                w = wk.tile([P, NBMAX * H1], bf16, tag="w")
                nc.scalar.activation(w[:, 0:nbi * H1], ee[:, 0:nbi * H1],
                                     Act.Exp)
                rhs_all = wk.tile([P, NBMAX, 264], bf16, tag="rhsall")
                nc.scalar.copy(
                    out=rhs_all[:, 0:nbi, F_IN:264],
                    in_=w[:, 0:nbi * H1].rearrange("p (a h) -> p a h", h=H1))
                # (d,h) layout: inner dim h unit-stride on all operands -> 2x
                nc.vector.tensor_tensor(
                    out=rhs_all[:, 0:nbi, 0:F_IN]
                        .rearrange("p a (d h) -> p a d h", h=H1),
                    in0=G[:, 0:nbi, 0:F_IN]
                        .rearrange("p a (d h) -> p a d h", h=H1),
                    in1=w[:, 0:nbi * H1]
                        .rearrange("p (a h) -> p a h", h=H1)[:, :, None, :]
                        .to_broadcast([P, nbi, HD, H1]),
                    op=Alu.mult)
                acc = ps_agg.tile([P, 264], f32, space="PSUM", tag="agg")
                for c in range(nbi):
                    nc.tensor.matmul(acc[:], lhsT=ic[:, c, 0:P],
                                     rhs=rhs_all[:, c, :],
                                     start=(c == 0), stop=(c == nbi - 1))

                den = wk.tile([P, H1], f32, tag="den")
                nc.vector.tensor_scalar_max(den[:], acc[:, F_IN:264], 1e-30)
                rec = wk.tile([P, H1], f32, tag="rec")
                nc.vector.reciprocal(rec[:], den[:])
                h = wk.tile([P, F_IN], f32, tag="h")
                nc.vector.tensor_tensor(
                    out=h[:].rearrange("p (d h) -> p d h", h=H1),
                    in0=acc[:, 0:F_IN].rearrange("p (d h) -> p d h", h=H1),
                    in1=rec[:, None, :].to_broadcast([P, HD, H1]),
                    op=Alu.mult)
                hb = wk.tile([P, F_IN], f32, tag="hb")
                nc.vector.tensor_tensor(out=hb[:], in0=h[:], in1=b1B[:],
                                        op=Alu.add)
                nc.scalar.activation(hb[:], hb[:], Act.Relu)

                f2 = ps_f2.tile([P, 42], f32, space="PSUM", tag="f2")
                for j in range(2):
                    ht_ps = ps_t.tile([P, 272], f32, space="PSUM", tag="htp")
                    nc.tensor.transpose(ht_ps[:, 0:P], hb[:, j * P:(j + 1) * P],
                                        ident[:])
                    ht = wk.tile([P, P], bf16, tag="ht")
                    nc.vector.tensor_copy(out=ht[:], in_=ht_ps[:, 0:P])
                    nc.tensor.matmul(f2[:], lhsT=ht[:], rhs=Wc2[:, j, :],
                                     start=(j == 0), stop=(j == 1))
                t2r = wk.tile([P, F2R], bf16, tag="t2r")
                nc.vector.tensor_copy(out=t2r[:, 0:41], in_=f2[:, 0:41])
                nc.vector.tensor_copy(out=er2_sb[:, b:b + 1], in_=f2[:, 41:42])
                sl = slice(b * P, (b + 1) * P)
                if b < ABLOCKS:
                    nc.sync.dma_start(T2_localA[sl, :], t2r[:])
                else:
                    sl2 = slice((b - ABLOCKS) * P, (b - ABLOCKS + 1) * P)
                    nc.sync.dma_start(T2_localB[sl2, :], t2r[:])
                ccol += nbi
                if b == ABLOCKS - 1:
                    allgather(T2_localA, T2A)
            allgather(T2_localB, T2B)

            # ---- layer 2 edge phase ----
            ccol = 0
            icol = 0
            for b in range(BLOCKS_PER_CORE):
                nbi = int(nb_tot[b])
                na, nbk = int(n_a[b]), int(n_b[b])
                G2 = gp.tile([P, NBMAX, F2R], bf16, tag="g2")
                icol = emit_gathers(G2, T2A[:], T2B[:], na, nbk, icol, F2R)

                ic = indp.tile([P, NBMAX, 2 * P], f8, tag="ind")
                nc.sync.dma_start(ic[:, 0:nbi, :],
                                  t_ind[:, ccol * 2 * P:(ccol + nbi) * 2 * P])

                ers = ps_er.tile([P, NBH], f32, space="PSUM", tag="ers")
                for c in range(nbi):
                    nc.tensor.matmul(ers[:, c:c + 1],
                                     lhsT=ic[:, c, P:2 * P],
                                     rhs=er2_sb[:, b:b + 1],
                                     start=True, stop=True)
                ee = wk.tile([P, NBMAX], f32, tag="ee2")
                nc.vector.tensor_tensor(
                    out=ee[:, 0:nbi],
                    in0=G2[:, 0:nbi, C2:C2 + 1].rearrange("p a b -> p (a b)"),
                    in1=ers[:, 0:nbi], op=Alu.add)
                nc.scalar.activation(ee[:, 0:nbi], ee[:, 0:nbi], Act.Prelu,
                                     alpha=alpha[:, :1])
                w2 = wk.tile([P, NBMAX], bf16, tag="w2")
                nc.scalar.activation(w2[:, 0:nbi], ee[:, 0:nbi], Act.Exp)

                rhs2 = wk.tile([P, NBMAX, 41], bf16, tag="rhs2a")
                nc.vector.tensor_tensor(
                    out=rhs2[:, 0:nbi, 0:C2],
                    in0=G2[:, 0:nbi, 0:C2],
                    in1=w2[:, 0:nbi, None].to_broadcast([P, nbi, C2]),
                    op=Alu.mult)
                nc.scalar.copy(out=rhs2[:, 0:nbi, C2:41],
                               in_=w2[:, 0:nbi, None])
                acc = ps_agg.tile([P, 264], f32, space="PSUM", tag="agg")
                for c in range(nbi):
                    nc.tensor.matmul(acc[:, 0:41], lhsT=ic[:, c, 0:P],
                                     rhs=rhs2[:, c, :],
                                     start=(c == 0), stop=(c == nbi - 1))

                den = wk.tile([P, 1], f32, tag="den2")
                nc.vector.tensor_scalar_max(den[:], acc[:, C2:41], 1e-30)
                rec = wk.tile([P, 1], f32, tag="rec2")
                nc.vector.reciprocal(rec[:], den[:])
                o = wk.tile([P, C2], f32, tag="o")
                nc.vector.tensor_tensor(
                    out=o[:], in0=acc[:, 0:C2],
                    in1=rec[:, :1].to_broadcast([P, C2]), op=Alu.mult)
                nc.vector.tensor_tensor(out=o[:], in0=o[:], in1=b2B[:],
                                        op=Alu.add)
                nc.sync.dma_start(t_out[b * P:(b + 1) * P, :], o[:])
                ccol += nbi

    nc.compile()
    return nc


def kernel(features, src, dst, W1, attn_l1, attn_r1, b1, W2, attn_l2,
           attn_r2, b2):
    from concourse import bass_utils

    features = np.asarray(features, np.float32)
    src = np.asarray(src)
    dst = np.asarray(dst)
    W1 = np.asarray(W1, np.float32)
    attn_l1 = np.asarray(attn_l1, np.float32)
    attn_r1 = np.asarray(attn_r1, np.float32)
    b1 = np.asarray(b1, np.float32)
    W2 = np.asarray(W2, np.float32)
    attn_l2 = np.asarray(attn_l2, np.float32)
    attn_r2 = np.asarray(attn_r2, np.float32)
    b2 = np.asarray(b2, np.float32)

    g = _prep_graph(src, dst)
    per_core, NBMAX, CTOT = _build_core_inputs(
        g, features, W1, attn_l1, attn_r1, W2, attn_l2, attn_r2, b1, b2)

    IDXCOLS = per_core[0]['idx'].shape[1]
    nc = _build_program(g, NBMAX, CTOT, IDXCOLS)

    in_maps = []
    for pc in per_core:
        in_maps.append({
            "xT": pc['xT'], "idx": pc['idx'], "indc": pc['indc'],
            "Wcat1": pc['Wcat1'], "Wcat2": pc['Wcat2'],
            "b1B": pc['b1B'], "b2B": pc['b2B'],
        })

    res = bass_utils.run_bass_kernel_spmd(
        nc, in_maps, core_ids=list(range(NCORES)),
        trace=bool(int(os.environ.get('KTRACE', '0'))))
    kernel.last_result = res

    out = np.zeros((N_NODES, C2), np.float32)
    for c in range(NCORES):
        oc = res.results[c]["out2"]
        for i in range(BLOCKS_PER_CORE):
            b = g['blocks_at'][c][i]
            lo = b * P
            hi = min(lo + P, N_NODES)
            if hi > lo:
                out[lo:hi] = oc[i * P: i * P + (hi - lo)]
    return out


kernel.last_result = None
